# revision 12
# baseline (speedup 1.0000x reference)
"""Trainium2 Bass kernel for nn_CAC_42511586296007 (circular-mask max-pool descriptor).

Reference (per batch b, channel c):
  v = l2norm_c(max_hw(x)) + sum over 153 circular masks m of l2norm_c(max_hw(x*m))
Masks: center + per-quadrant rings/circles of integer radius on 28x28.

v5 design (host-side stream layout, t-major, DVE-lean):
  - batch sharded 8 ways (4/core), channels on partitions (128p x 8t tiles).
  - Host (layout only: permute/duplicate/pad/cast): per batch, emit the
    ring-sorted stream bf16 [128p, u(8) x t(8) x Q(163)]: u fold slots, each
    holding m_s blocks per segment (ring x quadrant + center/outer), padded
    cyclically; invalid quadrant cells = -inf. t-major keeps every on-chip
    op unit-stride in its innermost dim.
  - Per batch: 2 half DMAs -> 4 in-place TT-max folds (bf16 2x, contiguous)
    -> class stage (2 copies + 3 grouped reduce_max, unit-inner) ->
    seg[b, t, k78] (k: 0 center, 1..76 ring-quad, 77 outer).
  - Phase 2 per half (2 batches): full max (unit-inner reduce); relu on
    ScalarE; circles via in-place Hillis-Steele prefix max (unit-inner);
    squares (ScalarE); channel norms via PE; Rsqrt (ScalarE); PE broadcast;
    scale + add + reduce_sum.
"""

import numpy as np

_B, _C, _HH, _WW = 32, 1024, 28, 28
_S = _HH * _WW
_NCORES = 8
_BL = _B // _NCORES       # 4 batches per core
_CT = _C // 128           # 8 channel tiles
_NSEG = 78                # center + 76 ring-quad + outer
_NSLOT = 158              # full,center,rings(2:78),center-dups(78:82),circles(82:158)
_NEG = -3.0e38
_QUADS = [(1, 1), (-1, 1), (1, -1), (-1, -1)]   # (sign_x, sign_y) ref order
_U = 8                    # stream fold slots
_Q = 163                  # stream blocks per u-slot
_NB = _U * _Q * _CT       # stream elems per partition (10432)


def _build_tables():
    """Stream position -> flat pixel id (0..783) or -1 (invalid -> -inf).
    Stream layout: [u(8), t(8), Q(163)] per partition; pixel table is per
    (u, pos) (t handled by the host gather)."""
    ij = np.arange(15)
    I, J = np.meshgrid(ij, ij, indexing="ij")
    RING = np.ceil(np.sqrt(I * I + J * J)).astype(int)
    cells_r = {r: [(i, j) for i in range(15) for j in range(15) if RING[i, j] == r]
               for r in range(1, 20)}
    outer_cells = [(i, j) for i in range(15) for j in range(15) if RING[i, j] >= 20]

    segs = []  # (entries list[(q,i,j)], m)
    segs.append(([(q, 0, 0) for q in range(4)], 1))            # center -> k0
    for r in list(range(1, 5)) + [18, 19]:                      # m=1 segs
        for q in range(4):
            segs.append(([(q, i, j) for (i, j) in cells_r[r]], 1))
    for r in range(5, 10):                                      # m=2
        for q in range(4):
            segs.append(([(q, i, j) for (i, j) in cells_r[r]], 2))
    for r in range(10, 18):                                     # m=3
        for q in range(4):
            segs.append(([(q, i, j) for (i, j) in cells_r[r]], 3))
    segs.append(([(q, i, j) for q in range(4) for (i, j) in outer_cells], 2))
    assert sum(m for _, m in segs) == _Q

    def pix(q, i, j):
        sx, sy = _QUADS[q]
        if (sy == 1 and i > 13) or (sx == 1 and j > 13):
            return -1
        return (14 + sy * i) * 28 + (14 + sx * j)

    pos_pix = np.zeros((_U, _Q), dtype=np.int64)
    for u in range(_U):
        off = 0
        for entries, m in segs:
            need = _U * m
            padded = [entries[k % len(entries)] for k in range(need)]
            for k in range(m):
                pos_pix[u, off + k] = pix(*padded[u * m + k])
            off += m
    return pos_pix


_POSPIX = _build_tables()          # [8, 163] pixel ids, -1 = -inf slot
_NC_CACHE = None
_BF16 = None


def _bf16():
    global _BF16
    if _BF16 is None:
        import ml_dtypes
        _BF16 = ml_dtypes.bfloat16
    return _BF16


def _host_stream(x):
    """x [32, 1024, 28, 28] f32 -> xg [8cores, BL, 128, NB] bf16,
    stream layout per partition = [u(8), t(8), Q(163)]."""
    xr = x.reshape(_B, _CT, 128, _S).transpose(0, 2, 1, 3)      # [B,128,t,784]
    pad = np.full((_B, 128, _CT, 1), _NEG, np.float32)
    xp = np.concatenate([xr, pad], axis=3)                      # [B,128,t,785]
    pixi = np.where(_POSPIX < 0, _S, _POSPIX).reshape(-1)       # [8*163]
    gs = xp[:, :, :, pixi]                                      # [B,128,t,u*Q]
    gs = gs.reshape(_B, 128, _CT, _U, _Q).transpose(0, 1, 3, 2, 4)  # [B,128,u,t,Q]
    gs = np.ascontiguousarray(gs.reshape(_B, 128, _NB)).astype(_bf16())
    return gs.reshape(_NCORES, _BL, 128, _NB)


def _build_nc():
    import concourse.bacc as bacc
    import concourse.mybir as mybir
    from concourse.tile import TileContext

    f32 = mybir.dt.float32
    bf16 = mybir.dt.bfloat16
    AX = mybir.AxisListType
    AF = mybir.ActivationFunctionType
    MAX = mybir.AluOpType.max
    MULT = mybir.AluOpType.mult
    ADD = mybir.AluOpType.add

    QT = _Q * _CT             # one u-slot (1304 elems)

    nc = bacc.Bacc("TRN2")
    xg = nc.dram_tensor("xg", [_BL, 128, _NB], bf16, kind="ExternalInput")
    out_d = nc.dram_tensor("out", [128, _BL * _CT], f32, kind="ExternalOutput")

    with TileContext(nc) as tc:
        with (
            tc.tile_pool(name="const", bufs=1) as cpool,
            tc.tile_pool(name="g", bufs=3) as gpool,
            tc.tile_pool(name="big", bufs=1) as bpool,
            tc.tile_pool(name="sm", bufs=2) as smpool,
            tc.tile_pool(name="psn", bufs=2, space="PSUM") as ppool_n,
            tc.tile_pool(name="pbc", bufs=2, space="PSUM") as ppool_b,
        ):
            ones_b = cpool.tile([128, 1], bf16, tag="ones_b")
            nc.vector.memset(ones_b[:], 1.0)
            ones1 = cpool.tile([1, 128], f32, tag="ones1")
            nc.vector.memset(ones1[:], 1.0)
            warm = cpool.tile([1, 8], f32, tag="warm")
            nc.vector.memset(warm[:], 1.0)
            nc.scalar.activation(out=warm[:], in_=warm[:], func=AF.Square)
            nc.scalar.activation(out=warm[:], in_=warm[:], func=AF.Sqrt)

            # seg/vt: [p, b(2), t(8), k] bf16, t-major
            seg_h = [bpool.tile([128, 2 * _CT * _NSEG], bf16, tag=f"seg{h}",
                                name=f"seg{h}") for h in range(2)]
            seg_hv = [t[:].rearrange("p (b t k) -> p b t k", b=2, t=_CT)
                      for t in seg_h]
            vt_h = [bpool.tile([128, 2 * _CT * _NSLOT], bf16, tag=f"vt{h}",
                               name=f"vt{h}") for h in range(2)]
            vt_hv = [t[:].rearrange("p (b t k) -> p b t k", b=2, t=_CT)
                     for t in vt_h]
            outv = cpool.tile([128, _BL * _CT], f32, tag="outv")

            def do_batch(b):
                g = gpool.tile([128, _NB], bf16, tag="g")
                H, Qt4, Qt2 = _NB // 2, _NB // 4, _NB // 8
                if b == 0:
                    # eighth DMAs + finest fold tree for fastest pipeline fill
                    for qq in range(8):
                        nc.sync.dma_start(
                            out=g[:, qq * Qt2:(qq + 1) * Qt2],
                            in_=xg[b, :, qq * Qt2:(qq + 1) * Qt2])
                    for qq in range(4):
                        o = 2 * qq * Qt2
                        nc.vector.tensor_tensor(
                            out=g[:, o:o + Qt2], in0=g[:, o:o + Qt2],
                            in1=g[:, o + Qt2:o + 2 * Qt2], op=MAX)
                    nc.vector.tensor_tensor(
                        out=g[:, 0:Qt2], in0=g[:, 0:Qt2],
                        in1=g[:, Qt4:Qt4 + Qt2], op=MAX)
                    nc.vector.tensor_tensor(
                        out=g[:, H:H + Qt2], in0=g[:, H:H + Qt2],
                        in1=g[:, H + Qt4:H + Qt4 + Qt2], op=MAX)
                    nc.vector.tensor_tensor(
                        out=g[:, 0:Qt2], in0=g[:, 0:Qt2],
                        in1=g[:, H:H + Qt2], op=MAX)
                elif b == 1:
                    # quarter DMAs
                    for qq in range(4):
                        nc.sync.dma_start(
                            out=g[:, qq * Qt4:(qq + 1) * Qt4],
                            in_=xg[b, :, qq * Qt4:(qq + 1) * Qt4])
                    for qq in range(4):
                        o = qq * Qt4
                        nc.vector.tensor_tensor(
                            out=g[:, o:o + Qt2], in0=g[:, o:o + Qt2],
                            in1=g[:, o + Qt2:o + Qt4], op=MAX)
                    nc.vector.tensor_tensor(
                        out=g[:, 0:Qt2], in0=g[:, 0:Qt2],
                        in1=g[:, Qt4:Qt4 + Qt2], op=MAX)
                    nc.vector.tensor_tensor(
                        out=g[:, H:H + Qt2], in0=g[:, H:H + Qt2],
                        in1=g[:, H + Qt4:H + Qt4 + Qt2], op=MAX)
                    nc.vector.tensor_tensor(
                        out=g[:, 0:Qt2], in0=g[:, 0:Qt2],
                        in1=g[:, H:H + Qt2], op=MAX)
                else:
                    nc.sync.dma_start(out=g[:, 0:H], in_=xg[b, :, 0:H])
                    nc.sync.dma_start(out=g[:, H:_NB], in_=xg[b, :, H:_NB])
                    # fold tree: u8 -> u4 -> u2 -> u1 (in place, contiguous)
                    nc.vector.tensor_tensor(
                        out=g[:, 0:Qt4], in0=g[:, 0:Qt4], in1=g[:, Qt4:H], op=MAX)
                    nc.vector.tensor_tensor(
                        out=g[:, H:H + Qt4], in0=g[:, H:H + Qt4],
                        in1=g[:, H + Qt4:_NB], op=MAX)
                    nc.vector.tensor_tensor(
                        out=g[:, 0:Qt4], in0=g[:, 0:Qt4], in1=g[:, H:H + Qt4],
                        op=MAX)
                    nc.vector.tensor_tensor(
                        out=g[:, 0:Qt2], in0=g[:, 0:Qt2], in1=g[:, Qt2:Qt4],
                        op=MAX)

                h, bl = b // 2, b % 2
                segv = seg_hv[h]
                # f = g[:, 0:QT] viewed [t(8), Q(163)]
                f = g[:, 0:QT].rearrange("p (t B) -> p t B", t=_CT)
                # m=1 blocks: center + r1-4 -> k0..16; r18-19 -> k69..76
                nc.vector.tensor_copy(
                    out=segv[:, bl, :, 0:17], in_=f[:, :, 0:17])
                nc.vector.tensor_copy(
                    out=segv[:, bl, :, 69:77], in_=f[:, :, 17:25])
                # m=2 class: r5-9 (20 segs) -> k17..36 (pair TT)
                f2 = f[:, :, 25:65].rearrange("p t (G m) -> p t G m", m=2)
                nc.vector.tensor_tensor(
                    out=segv[:, bl, :, 17:37], in0=f2[:, :, :, 0],
                    in1=f2[:, :, :, 1], op=MAX)
                # m=3 class: r10-17 (32 segs) -> k37..68 (two TTs)
                f3 = f[:, :, 65:161].rearrange("p t (G m) -> p t G m", m=3)
                nc.vector.tensor_tensor(
                    out=segv[:, bl, :, 37:69], in0=f3[:, :, :, 0],
                    in1=f3[:, :, :, 1], op=MAX)
                nc.vector.tensor_tensor(
                    out=segv[:, bl, :, 37:69], in0=segv[:, bl, :, 37:69],
                    in1=f3[:, :, :, 2], op=MAX)
                # outer -> k77
                nc.vector.tensor_tensor(
                    out=segv[:, bl, :, 77:78], in0=f[:, :, 161:162],
                    in1=f[:, :, 162:163], op=MAX)

            def do_phase2(h, b0, nb):
                segv = seg_hv[h][:, b0:b0 + nb]
                vtv = vt_hv[h][:, b0:b0 + nb]
                # relu: center + rings (ScalarE)
                nc.scalar.activation(
                    out=vtv[:, :, :, 1:78], in_=segv[:, :, :, 0:77],
                    func=AF.Relu)
                # circles: chain = [center x4 | rings]; Hillis prefix in place
                nc.vector.tensor_copy(
                    out=vtv[:, :, :, 82:158], in_=vtv[:, :, :, 2:78])
                nc.vector.tensor_copy(
                    out=vtv[:, :, :, 78:82],
                    in_=vtv[:, :, :, 1:2].broadcast_to((128, nb, _CT, 4)))
                for s in (1, 2, 4, 8, 16):
                    nc.vector.tensor_tensor(
                        out=vtv[:, :, :, 78 + 4 * s:158],
                        in0=vtv[:, :, :, 78 + 4 * s:158],
                        in1=vtv[:, :, :, 78:158 - 4 * s], op=MAX)
                # full max = max(circle_19 over q, outer) -> slot 0
                nc.vector.reduce_max(
                    out=vtv[:, :, :, 0:1], in_=vtv[:, :, :, 154:158],
                    axis=AX.X)
                nc.vector.tensor_tensor(
                    out=vtv[:, :, :, 0:1], in0=vtv[:, :, :, 0:1],
                    in1=segv[:, :, :, 77:78], op=MAX)
                # squares (ScalarE) + channel-norm matmuls (PE)
                sq = smpool.tile([128, nb * _CT * _NSLOT], bf16, tag=f"sq{nb}")
                nc.scalar.activation(
                    out=sq[:], in_=vtv.rearrange("p b t k -> p (b t k)"),
                    func=AF.Square)
                sq_v = sq[:].rearrange("p (b t k) -> p b t k", b=nb, t=_CT)
                nrm = smpool.tile([1, nb * _NSLOT], f32, tag=f"nrm{nb}")
                inv = smpool.tile([1, nb * _NSLOT], f32, tag=f"inv{nb}")
                ps = ppool_n.tile([1, nb * _NSLOT], f32, tag=f"psn{nb}")
                ps_v = ps[:].rearrange("p (b k) -> p b k", b=nb)
                for ct in range(_CT):
                    nc.tensor.matmul(
                        ps_v, ones_b[:], sq_v[:, :, ct, :],
                        start=(ct == 0), stop=(ct == _CT - 1))
                nc.scalar.activation(out=nrm[:], in_=ps[:], func=AF.Sqrt)
                nc.vector.reciprocal_approx_fast(out=inv[:], in_=nrm[:])
                pb = ppool_b.tile([128, nb * _NSLOT], f32, tag=f"pbc{nb}")
                nc.tensor.matmul(pb[:], ones1[:], inv[:], start=True, stop=True)
                pbs = smpool.tile([128, nb * _NSLOT], bf16, tag=f"pbs{nb}")
                nc.scalar.activation(out=pbs[:], in_=pb[:], func=AF.Copy)
                # scale by 1/norm (broadcast over t; unit-inner k)
                scr = smpool.tile([128, nb * _CT * _NSLOT], bf16, tag=f"scr{nb}")
                scr_v = scr[:].rearrange("p (b t k) -> p b t k", b=nb, t=_CT)
                nc.vector.tensor_tensor(
                    out=scr_v, in0=vtv,
                    in1=pbs[:].rearrange("p (b k) -> p b k", b=nb)[
                        :, :, None, :].broadcast_to((128, nb, _CT, _NSLOT)),
                    op=MULT)
                nc.vector.tensor_tensor(
                    out=scr_v[:, :, :, 2:78], in0=scr_v[:, :, :, 2:78],
                    in1=scr_v[:, :, :, 82:158], op=ADD)
                nc.vector.reduce_sum(
                    out=outv[:, (2 * h + b0) * _CT:(2 * h + b0 + nb) * _CT],
                    in_=scr_v[:, :, :, 0:78], axis=AX.X)

            do_batch(0)
            do_batch(1)
            do_phase2(0, 0, 2)
            do_batch(2)
            do_phase2(1, 0, 1)
            do_batch(3)
            do_phase2(1, 1, 1)
            nc.sync.dma_start(out=out_d[:], in_=outv[:])

    nc.finalize()
    return nc


def _get_nc():
    global _NC_CACHE
    if _NC_CACHE is None:
        _NC_CACHE = _build_nc()
    return _NC_CACHE


def _run(x, trace=False):
    from concourse.bass_utils import run_bass_kernel_spmd

    nc = _get_nc()
    x = np.ascontiguousarray(np.asarray(x, dtype=np.float32))
    xg = _host_stream(x)
    in_maps = [{"xg": np.ascontiguousarray(xg[c])} for c in range(_NCORES)]
    res = run_bass_kernel_spmd(
        nc, in_maps, core_ids=list(range(_NCORES)), trace=trace)
    out = np.empty((_B, _C), np.float32)
    for c in range(_NCORES):
        r = np.asarray(res.results[c]["out"])            # [128, 32]
        rr = r.reshape(128, _BL, _CT)                    # [p, b, ct]
        out[c * _BL:(c + 1) * _BL] = rr.transpose(1, 2, 0).reshape(_BL, _C)
    return out.reshape(_B, _C, 1, 1), res


def kernel(x):
    out, _ = _run(x, trace=False)
    return out


# revision 13
# speedup vs baseline: 1.0304x; 1.0304x over previous
"""Trainium2 Bass kernel for nn_CAC_42511586296007 (circular-mask max-pool descriptor).

Reference (per batch b, channel c):
  v = l2norm_c(max_hw(x)) + sum over 153 circular masks m of l2norm_c(max_hw(x*m))
Masks: center + per-quadrant rings/circles of integer radius on 28x28.

v5 design (host-side stream layout, t-major, DVE-lean):
  - batch sharded 8 ways (4/core), channels on partitions (128p x 8t tiles).
  - Host (layout only: permute/duplicate/pad/cast): per batch, emit the
    ring-sorted stream bf16 [128p, u(8) x t(8) x Q(163)]: u fold slots, each
    holding m_s blocks per segment (ring x quadrant + center/outer), padded
    cyclically; invalid quadrant cells = -inf. t-major keeps every on-chip
    op unit-stride in its innermost dim.
  - Per batch: 2 half DMAs -> 4 in-place TT-max folds (bf16 2x, contiguous)
    -> class stage (2 copies + 3 grouped reduce_max, unit-inner) ->
    seg[b, t, k78] (k: 0 center, 1..76 ring-quad, 77 outer).
  - Phase 2 per half (2 batches): full max (unit-inner reduce); relu on
    ScalarE; circles via in-place Hillis-Steele prefix max (unit-inner);
    squares (ScalarE); channel norms via PE; Rsqrt (ScalarE); PE broadcast;
    scale + add + reduce_sum.
"""

import numpy as np

_B, _C, _HH, _WW = 32, 1024, 28, 28
_S = _HH * _WW
_NCORES = 8
_BL = _B // _NCORES       # 4 batches per core
_CT = _C // 128           # 8 channel tiles
_NSEG = 78                # center + 76 ring-quad + outer
_NSLOT = 158              # full,center,rings(2:78),center-dups(78:82),circles(82:158)
_NEG = -3.0e38
_QUADS = [(1, 1), (-1, 1), (1, -1), (-1, -1)]   # (sign_x, sign_y) ref order
_U = 8                    # stream fold slots
_Q = 163                  # stream blocks per u-slot
_NB = _U * _Q * _CT       # stream elems per partition (10432)


def _build_tables():
    """Stream position -> flat pixel id (0..783) or -1 (invalid -> -inf).
    Stream layout: [u(8), t(8), Q(163)] per partition; pixel table is per
    (u, pos) (t handled by the host gather)."""
    ij = np.arange(15)
    I, J = np.meshgrid(ij, ij, indexing="ij")
    RING = np.ceil(np.sqrt(I * I + J * J)).astype(int)
    cells_r = {r: [(i, j) for i in range(15) for j in range(15) if RING[i, j] == r]
               for r in range(1, 20)}
    outer_cells = [(i, j) for i in range(15) for j in range(15) if RING[i, j] >= 20]

    segs = []  # (entries list[(q,i,j)], m)
    segs.append(([(q, 0, 0) for q in range(4)], 1))            # center -> k0
    for r in list(range(1, 5)) + [18, 19]:                      # m=1 segs
        for q in range(4):
            segs.append(([(q, i, j) for (i, j) in cells_r[r]], 1))
    for r in range(5, 10):                                      # m=2
        for q in range(4):
            segs.append(([(q, i, j) for (i, j) in cells_r[r]], 2))
    for r in range(10, 18):                                     # m=3
        for q in range(4):
            segs.append(([(q, i, j) for (i, j) in cells_r[r]], 3))
    segs.append(([(q, i, j) for q in range(4) for (i, j) in outer_cells], 2))
    assert sum(m for _, m in segs) == _Q

    def pix(q, i, j):
        sx, sy = _QUADS[q]
        if (sy == 1 and i > 13) or (sx == 1 and j > 13):
            return -1
        return (14 + sy * i) * 28 + (14 + sx * j)

    pos_pix = np.zeros((_U, _Q), dtype=np.int64)
    for u in range(_U):
        off = 0
        for entries, m in segs:
            need = _U * m
            padded = [entries[k % len(entries)] for k in range(need)]
            for k in range(m):
                pos_pix[u, off + k] = pix(*padded[u * m + k])
            off += m
    return pos_pix


_POSPIX = _build_tables()          # [8, 163] pixel ids, -1 = -inf slot
_NC_CACHE = None
_BF16 = None


def _bf16():
    global _BF16
    if _BF16 is None:
        import ml_dtypes
        _BF16 = ml_dtypes.bfloat16
    return _BF16


def _host_stream(x):
    """x [32, 1024, 28, 28] f32 -> xg [8cores, BL, 128, NB] bf16,
    stream layout per partition = [u(8), t(8), Q(163)]."""
    xr = x.reshape(_B, _CT, 128, _S).transpose(0, 2, 1, 3)      # [B,128,t,784]
    pad = np.full((_B, 128, _CT, 1), _NEG, np.float32)
    xp = np.concatenate([xr, pad], axis=3)                      # [B,128,t,785]
    pixi = np.where(_POSPIX < 0, _S, _POSPIX).reshape(-1)       # [8*163]
    gs = xp[:, :, :, pixi]                                      # [B,128,t,u*Q]
    gs = gs.reshape(_B, 128, _CT, _U, _Q).transpose(0, 1, 3, 2, 4)  # [B,128,u,t,Q]
    gs = np.ascontiguousarray(gs.reshape(_B, 128, _NB)).astype(_bf16())
    return gs.reshape(_NCORES, _BL, 128, _NB)


def _build_nc():
    import concourse.bacc as bacc
    import concourse.mybir as mybir
    from concourse.tile import TileContext

    f32 = mybir.dt.float32
    bf16 = mybir.dt.bfloat16
    AX = mybir.AxisListType
    AF = mybir.ActivationFunctionType
    MAX = mybir.AluOpType.max
    MULT = mybir.AluOpType.mult
    ADD = mybir.AluOpType.add

    QT = _Q * _CT             # one u-slot (1304 elems)

    nc = bacc.Bacc("TRN2")
    xg = nc.dram_tensor("xg", [_BL, 128, _NB], bf16, kind="ExternalInput")
    out_d = nc.dram_tensor("out", [128, _BL * _CT], f32, kind="ExternalOutput")

    with TileContext(nc) as tc:
        with (
            tc.tile_pool(name="const", bufs=1) as cpool,
            tc.tile_pool(name="g", bufs=3) as gpool,
            tc.tile_pool(name="big", bufs=1) as bpool,
            tc.tile_pool(name="sm", bufs=2) as smpool,
            tc.tile_pool(name="psn", bufs=2, space="PSUM") as ppool_n,
            tc.tile_pool(name="pbc", bufs=2, space="PSUM") as ppool_b,
        ):
            ones_b = cpool.tile([128, 1], bf16, tag="ones_b")
            nc.vector.memset(ones_b[:], 1.0)
            ones1 = cpool.tile([1, 128], f32, tag="ones1")
            nc.vector.memset(ones1[:], 1.0)
            warm = cpool.tile([1, 8], f32, tag="warm")
            nc.vector.memset(warm[:], 1.0)
            nc.scalar.activation(out=warm[:], in_=warm[:], func=AF.Square)
            nc.scalar.activation(out=warm[:], in_=warm[:], func=AF.Sqrt)

            # seg/vt: [p, b(2), t(8), k] bf16, t-major
            seg_h = [bpool.tile([128, 2 * _CT * _NSEG], bf16, tag=f"seg{h}",
                                name=f"seg{h}") for h in range(2)]
            seg_hv = [t[:].rearrange("p (b t k) -> p b t k", b=2, t=_CT)
                      for t in seg_h]
            vt_h = [bpool.tile([128, 2 * _CT * _NSLOT], bf16, tag=f"vt{h}",
                               name=f"vt{h}") for h in range(2)]
            vt_hv = [t[:].rearrange("p (b t k) -> p b t k", b=2, t=_CT)
                     for t in vt_h]
            outv = cpool.tile([128, _BL * _CT], f32, tag="outv")

            def do_batch(b):
                g = gpool.tile([128, _NB], bf16, tag="g")
                H, Qt4, Qt2 = _NB // 2, _NB // 4, _NB // 8
                if b == 0:
                    # quarter DMAs + finer fold tree for a faster pipeline fill
                    for qq in range(4):
                        nc.sync.dma_start(
                            out=g[:, qq * Qt4:(qq + 1) * Qt4],
                            in_=xg[b, :, qq * Qt4:(qq + 1) * Qt4])
                    for qq in range(4):
                        o = qq * Qt4
                        nc.vector.tensor_tensor(
                            out=g[:, o:o + Qt2], in0=g[:, o:o + Qt2],
                            in1=g[:, o + Qt2:o + Qt4], op=MAX)
                    nc.vector.tensor_tensor(
                        out=g[:, 0:Qt2], in0=g[:, 0:Qt2],
                        in1=g[:, Qt4:Qt4 + Qt2], op=MAX)
                    nc.vector.tensor_tensor(
                        out=g[:, H:H + Qt2], in0=g[:, H:H + Qt2],
                        in1=g[:, H + Qt4:H + Qt4 + Qt2], op=MAX)
                    nc.vector.tensor_tensor(
                        out=g[:, 0:Qt2], in0=g[:, 0:Qt2],
                        in1=g[:, H:H + Qt2], op=MAX)
                else:
                    nc.sync.dma_start(out=g[:, 0:H], in_=xg[b, :, 0:H])
                    nc.sync.dma_start(out=g[:, H:_NB], in_=xg[b, :, H:_NB])
                    # fold tree: u8 -> u4 -> u2 -> u1 (in place, contiguous)
                    nc.vector.tensor_tensor(
                        out=g[:, 0:Qt4], in0=g[:, 0:Qt4], in1=g[:, Qt4:H], op=MAX)
                    nc.vector.tensor_tensor(
                        out=g[:, H:H + Qt4], in0=g[:, H:H + Qt4],
                        in1=g[:, H + Qt4:_NB], op=MAX)
                    nc.vector.tensor_tensor(
                        out=g[:, 0:Qt4], in0=g[:, 0:Qt4], in1=g[:, H:H + Qt4],
                        op=MAX)
                    nc.vector.tensor_tensor(
                        out=g[:, 0:Qt2], in0=g[:, 0:Qt2], in1=g[:, Qt2:Qt4],
                        op=MAX)

                h, bl = b // 2, b % 2
                segv = seg_hv[h]
                # f = g[:, 0:QT] viewed [t(8), Q(163)]
                f = g[:, 0:QT].rearrange("p (t B) -> p t B", t=_CT)
                # m=1 blocks: center + r1-4 -> k0..16; r18-19 -> k69..76
                nc.vector.tensor_copy(
                    out=segv[:, bl, :, 0:17], in_=f[:, :, 0:17])
                nc.vector.tensor_copy(
                    out=segv[:, bl, :, 69:77], in_=f[:, :, 17:25])
                # m=2 class: r5-9 (20 segs) -> k17..36 (pair TT)
                f2 = f[:, :, 25:65].rearrange("p t (G m) -> p t G m", m=2)
                nc.vector.tensor_tensor(
                    out=segv[:, bl, :, 17:37], in0=f2[:, :, :, 0],
                    in1=f2[:, :, :, 1], op=MAX)
                # m=3 class: r10-17 (32 segs) -> k37..68 (two TTs)
                f3 = f[:, :, 65:161].rearrange("p t (G m) -> p t G m", m=3)
                nc.vector.tensor_tensor(
                    out=segv[:, bl, :, 37:69], in0=f3[:, :, :, 0],
                    in1=f3[:, :, :, 1], op=MAX)
                nc.vector.tensor_tensor(
                    out=segv[:, bl, :, 37:69], in0=segv[:, bl, :, 37:69],
                    in1=f3[:, :, :, 2], op=MAX)
                # outer -> k77
                nc.vector.tensor_tensor(
                    out=segv[:, bl, :, 77:78], in0=f[:, :, 161:162],
                    in1=f[:, :, 162:163], op=MAX)

            def do_phase2(h, b0, nb):
                segv = seg_hv[h][:, b0:b0 + nb]
                vtv = vt_hv[h][:, b0:b0 + nb]
                # relu: center + rings (ScalarE)
                nc.scalar.activation(
                    out=vtv[:, :, :, 1:78], in_=segv[:, :, :, 0:77],
                    func=AF.Relu)
                # circles: chain = [center x4 | rings]; Hillis prefix in place
                nc.vector.tensor_copy(
                    out=vtv[:, :, :, 82:158], in_=vtv[:, :, :, 2:78])
                nc.vector.tensor_copy(
                    out=vtv[:, :, :, 78:82],
                    in_=vtv[:, :, :, 1:2].broadcast_to((128, nb, _CT, 4)))
                for s in (1, 2, 4, 8, 16):
                    nc.vector.tensor_tensor(
                        out=vtv[:, :, :, 78 + 4 * s:158],
                        in0=vtv[:, :, :, 78 + 4 * s:158],
                        in1=vtv[:, :, :, 78:158 - 4 * s], op=MAX)
                # full max = max(circle_19 over q, outer) -> slot 0
                nc.vector.reduce_max(
                    out=vtv[:, :, :, 0:1], in_=vtv[:, :, :, 154:158],
                    axis=AX.X)
                nc.vector.tensor_tensor(
                    out=vtv[:, :, :, 0:1], in0=vtv[:, :, :, 0:1],
                    in1=segv[:, :, :, 77:78], op=MAX)
                # squares (ScalarE) + channel-norm matmuls (PE)
                sq = smpool.tile([128, nb * _CT * _NSLOT], bf16, tag=f"sq{nb}")
                nc.scalar.activation(
                    out=sq[:], in_=vtv.rearrange("p b t k -> p (b t k)"),
                    func=AF.Square)
                sq_v = sq[:].rearrange("p (b t k) -> p b t k", b=nb, t=_CT)
                nrm = smpool.tile([1, nb * _NSLOT], f32, tag=f"nrm{nb}")
                inv = smpool.tile([1, nb * _NSLOT], f32, tag=f"inv{nb}")
                ps = ppool_n.tile([1, nb * _NSLOT], f32, tag=f"psn{nb}")
                ps_v = ps[:].rearrange("p (b k) -> p b k", b=nb)
                for ct in range(_CT):
                    nc.tensor.matmul(
                        ps_v, ones_b[:], sq_v[:, :, ct, :],
                        start=(ct == 0), stop=(ct == _CT - 1))
                nc.scalar.activation(out=nrm[:], in_=ps[:], func=AF.Sqrt)
                nc.vector.reciprocal_approx_fast(out=inv[:], in_=nrm[:])
                pb = ppool_b.tile([128, nb * _NSLOT], f32, tag=f"pbc{nb}")
                nc.tensor.matmul(pb[:], ones1[:], inv[:], start=True, stop=True)
                pbs = smpool.tile([128, nb * _NSLOT], bf16, tag=f"pbs{nb}")
                nc.scalar.activation(out=pbs[:], in_=pb[:], func=AF.Copy)
                # scale by 1/norm (broadcast over t; unit-inner k)
                scr = smpool.tile([128, nb * _CT * _NSLOT], bf16, tag=f"scr{nb}")
                scr_v = scr[:].rearrange("p (b t k) -> p b t k", b=nb, t=_CT)
                nc.vector.tensor_tensor(
                    out=scr_v, in0=vtv,
                    in1=pbs[:].rearrange("p (b k) -> p b k", b=nb)[
                        :, :, None, :].broadcast_to((128, nb, _CT, _NSLOT)),
                    op=MULT)
                nc.vector.tensor_tensor(
                    out=scr_v[:, :, :, 2:78], in0=scr_v[:, :, :, 2:78],
                    in1=scr_v[:, :, :, 82:158], op=ADD)
                nc.vector.reduce_sum(
                    out=outv[:, (2 * h + b0) * _CT:(2 * h + b0 + nb) * _CT],
                    in_=scr_v[:, :, :, 0:78], axis=AX.X)

            do_batch(0)
            do_batch(1)
            do_phase2(0, 0, 2)
            do_batch(2)
            do_phase2(1, 0, 1)
            do_batch(3)
            do_phase2(1, 1, 1)
            nc.sync.dma_start(out=out_d[:], in_=outv[:])

    nc.finalize()
    return nc


def _get_nc():
    global _NC_CACHE
    if _NC_CACHE is None:
        _NC_CACHE = _build_nc()
    return _NC_CACHE


def _run(x, trace=False):
    from concourse.bass_utils import run_bass_kernel_spmd

    nc = _get_nc()
    x = np.ascontiguousarray(np.asarray(x, dtype=np.float32))
    xg = _host_stream(x)
    in_maps = [{"xg": np.ascontiguousarray(xg[c])} for c in range(_NCORES)]
    res = run_bass_kernel_spmd(
        nc, in_maps, core_ids=list(range(_NCORES)), trace=trace)
    out = np.empty((_B, _C), np.float32)
    for c in range(_NCORES):
        r = np.asarray(res.results[c]["out"])            # [128, 32]
        rr = r.reshape(128, _BL, _CT)                    # [p, b, ct]
        out[c * _BL:(c + 1) * _BL] = rr.transpose(1, 2, 0).reshape(_BL, _C)
    return out.reshape(_B, _C, 1, 1), res


def kernel(x):
    out, _ = _run(x, trace=False)
    return out
